# revision 1
# baseline (speedup 1.0000x reference)
"""Multi-head attention (b=4, n=2048, dim=1024, heads=16) on 8 TRN2 cores.

Sharding: tensor-parallel over heads (2 heads per core) + row-parallel output
projection; host sums the 8 partial outputs and adds the bias.

Per-core math (heads h0=2c, h1=2c+1):
  qkv^T = w_in_c^T @ x^T          (f32r matmuls, w stationary)
  S^T   = k_h^T.T @ q_h^T         (bf16, K=64, per-head partition halves)
  E^T   = exp(S^T / 8)            (ACT, no max subtraction: scores ~ N(0,1))
  [o^T; denom] = [v_h | 1].T @ E^T  (bf16, M=65 fuses softmax denominator)
  o_norm^T = o^T * (1/denom)      (DVE + DMA partition-broadcast)
  partial^T = w_out_c.T @ o_norm^T  (bf16, w stationary) -> DRAM
"""

import os
import sys
import types

import numpy as np

# NTFF-profile hook shim: container's antenv lacks axon_hooks; harmless if
# tracing is never requested.
if "antenv.axon_hooks" not in sys.modules:
    try:
        from trn_agent_boot.trn_boot import _ntff_profile_via_ctypes

        _m = types.ModuleType("antenv.axon_hooks")
        _h = _ntff_profile_via_ctypes("/opt/axon/libaxon_pjrt.so")
        _m.get_axon_ntff_profile_hook = lambda: _h
        _m.set_axon_ntff_profile_hook = lambda hook: None
        sys.modules["antenv.axon_hooks"] = _m
    except Exception:
        pass

import ml_dtypes

import concourse.bacc as bacc
import concourse.bass as bass
import concourse.mybir as mybir
import concourse.tile as tile
from concourse.bass_utils import run_bass_kernel_spmd
from concourse.masks import make_identity

F32 = mybir.dt.float32
F32R = mybir.dt.float32r
BF16 = mybir.dt.bfloat16

B, N, DIM, HEADS = 4, 2048, 1024, 16
HD = DIM // HEADS          # 64
NCORES = 8
HPC = HEADS // NCORES      # 2 heads per core
NT = B * N                 # 8192 tokens
MQKV = 3 * HPC * HD        # 384 qkv output dims per core
SCALE = HD ** -0.5         # 0.125

KT_TILES = DIM // 128      # 8 k-tiles in the projection contraction
NB = NT // 512             # 16 n-blocks in phase 1
JT = N // 128              # 16 j-tiles per batch
IH = N // 1024             # 2 i-halves per batch


def _build_nc():
    nc = bacc.Bacc("TRN2", target_bir_lowering=False, debug=False)

    xT = nc.dram_tensor("xT", [DIM, NT], BF16, kind="ExternalInput")
    w_in_c = nc.dram_tensor("w_in_c", [DIM, MQKV], BF16, kind="ExternalInput")
    w_out_c = nc.dram_tensor("w_out_c", [128, DIM], BF16, kind="ExternalInput")
    po = nc.dram_tensor("po", [DIM, NT], F32, kind="ExternalOutput")
    dn_dram = nc.dram_tensor("dn_dram", [16, 1024], F32)
    rc_dram = nc.dram_tensor("rc_dram", [16, 1024], F32)

    with tile.TileContext(nc) as tc:
        with (
            tc.tile_pool(name="big", bufs=1) as big,
            tc.tile_pool(name="strm", bufs=2) as strm,
            tc.tile_pool(name="et", bufs=4) as etp,
            tc.tile_pool(name="ps", bufs=2, space="PSUM") as ps,
        ):
            # ---- persistent SBUF ----
            QT = big.tile([128, NT], BF16)    # [q_h0(0:64); q_h1(64:128)]^T
            KT = big.tile([128, NT], BF16)
            Vt = big.tile([128, B * JT, 130], BF16)  # [v_h0|1|v_h1|1] per j-tile
            o_sb = big.tile([128, NT], BF16)  # o^T both heads (normed in place)
            w_in_sb = big.tile([128, KT_TILES, MQKV], BF16)
            w_out_sb = big.tile([128, DIM], BF16)

            nc.sync.dma_start(
                out=w_in_sb,
                in_=w_in_c.rearrange("(kt p) m -> p kt m", p=128),
            )
            nc.sync.dma_start(out=w_out_sb, in_=w_out_c[:, :])
            ident = big.tile([128, 128], BF16)
            make_identity(nc, ident)
            nc.vector.memset(Vt[:, :, 64], 1.0)
            nc.vector.memset(Vt[:, :, 129], 1.0)

            xT_r = xT.rearrange("(kt p) n -> p kt n", p=128)

            # ================= Phase 1: QKV projection =================
            NB1 = NT // 1024
            for nb in range(NB1):
                ncol = slice(nb * 1024, (nb + 1) * 1024)
                xin = strm.tile([128, KT_TILES, 2, 512], BF16, tag="xin")
                for k in range(KT_TILES):
                    nc.sync.dma_start(
                        out=xin[:, k, :, :],
                        in_=xT_r[:, k, ncol].rearrange(
                            "p (a b) -> p a b", b=512
                        ),
                    )
                for m in range(3):
                    pj = ps.tile(
                        [128, 2, 512], F32, tag="ps_s", name=f"pj{nb}_{m}"
                    )
                    for k in range(KT_TILES):
                        for a in range(2):
                            nc.tensor.matmul(
                                pj[:, a, :],
                                w_in_sb[:, k, m * 128:(m + 1) * 128],
                                xin[:, k, a, :],
                                start=(k == 0),
                                stop=(k == KT_TILES - 1),
                            )
                    pjf = pj.rearrange("p a b -> p (a b)")
                    if m == 0:
                        nc.vector.tensor_copy(QT[:, ncol], pjf)
                    elif m == 1:
                        nc.vector.tensor_copy(KT[:, ncol], pjf)
                    else:
                        vstage = strm.tile([128, 1024], BF16, tag="vstage")
                        nc.vector.tensor_copy(vstage, pjf)
                        for c in range(8):
                            g = nb * 8 + c
                            tp = ps.tile(
                                [128, 128], BF16, tag="ps_o", bufs=2, name=f"tp{g}"
                            )
                            nc.tensor.transpose(
                                tp, vstage[:, c * 128:(c + 1) * 128], ident
                            )
                            nc.vector.tensor_copy(Vt[:, g, 0:64], tp[:, 0:64])
                            nc.vector.tensor_copy(
                                Vt[:, g, 65:129], tp[:, 64:128]
                            )

            # ====== Phase 2+3: attention, per-chunk normalize + projection ==
            # chunk = (b, ihalf); denom rows in dn_dram at b*4 + ihalf*2 + h
            def emit_proj(bp, ihp):
                i0p = bp * N + ihp * 1024
                for mt in range(DIM // 128):
                    pp = ps.tile(
                        [128, 2, 512], F32, tag="ps_s", name=f"pp{bp}_{ihp}_{mt}"
                    )
                    for a in range(2):
                        nc.tensor.matmul(
                            pp[:, a, :],
                            w_out_sb[:, mt * 128:(mt + 1) * 128],
                            o_sb[:, i0p + a * 512:i0p + (a + 1) * 512],
                            start=True, stop=True,
                        )
                    pout = strm.tile([128, 1024], F32, tag="pout", bufs=4)
                    if mt % 2 == 0:
                        nc.vector.tensor_copy(
                            pout, pp.rearrange("p a b -> p (a b)")
                        )
                    else:
                        nc.scalar.copy(pout, pp.rearrange("p a b -> p (a b)"))
                    nc.sync.dma_start(
                        out=po[mt * 128:(mt + 1) * 128, i0p:i0p + 1024],
                        in_=pout,
                    )

            proj_ready = []
            for b in range(B):
                for ihalf in range(IH):
                    # flush a projection chunk whose normalize finished
                    # two chunks ago (so nothing here waits on it)
                    if len(proj_ready) >= 2:
                        emit_proj(*proj_ready.pop(0))

                    i0 = b * N + ihalf * 1024
                    icol = slice(i0, i0 + 1024)
                    po_h = [
                        ps.tile(
                            [65, 2, 512], F32, tag="ps_o", bufs=2,
                            name=f"po{b}_{ihalf}_{h}",
                        )
                        for h in range(HPC)
                    ]
                    ets = {}
                    for jt in range(JT + 1):
                        if jt < JT:
                            jcol = slice(
                                b * N + jt * 128, b * N + jt * 128 + 128
                            )
                            for h in range(HPC):
                                hp = slice(h * 64, (h + 1) * 64)
                                st = ps.tile(
                                    [128, 2, 512], F32, tag="ps_s",
                                    name=f"st{b}_{ihalf}_{jt}_{h}",
                                )
                                for a in range(2):
                                    nc.tensor.matmul(
                                        st[:, a, :], KT[hp, jcol],
                                        QT[hp, i0 + a * 512:i0 + (a + 1) * 512],
                                        start=True, stop=True,
                                    )
                                et = etp.tile(
                                    [128, 1024], BF16, tag="et", name="et"
                                )
                                nc.scalar.activation(
                                    et, st.rearrange("p a b -> p (a b)"),
                                    mybir.ActivationFunctionType.Exp,
                                    scale=SCALE,
                                )
                                ets[(jt, h)] = et
                        if jt > 0:
                            jp = jt - 1
                            for h in range(HPC):
                                et = ets.pop((jp, h))
                                for a in range(2):
                                    nc.tensor.matmul(
                                        po_h[h][:, a, :],
                                        Vt[:, b * JT + jp, h * 65:h * 65 + 65],
                                        et[:, a * 512:(a + 1) * 512],
                                        start=(jp == 0), stop=(jp == JT - 1),
                                    )
                    # drain psum
                    for h in range(HPC):
                        seg = b * 4 + ihalf * 2 + h
                        po_f = po_h[h].rearrange("p a b -> p (a b)")
                        if h == 0:
                            nc.vector.tensor_copy(
                                o_sb[0:64, icol], po_f[0:64, :]
                            )
                        else:
                            h1s = strm.tile([64, 1024], BF16, tag="h1s")
                            nc.vector.tensor_copy(h1s, po_f[0:64, :])
                            nc.sync.dma_start(out=o_sb[64:128, icol], in_=h1s)
                        dnst = strm.tile([1, 1024], F32, tag="dnst")
                        nc.vector.tensor_copy(dnst, po_f[64:65, :])
                        nc.sync.dma_start(
                            out=dn_dram[seg:seg + 1, :], in_=dnst[0:1, :]
                        )
                    # normalize this chunk (overlaps next chunk's compute)
                    g0 = b * 4 + ihalf * 2
                    dns = strm.tile([2, 1024], F32, tag="dns")
                    nc.sync.dma_start(out=dns, in_=dn_dram[g0:g0 + 2, :])
                    with nc.allow_low_precision(reason="denom broadcast"):
                        nc.vector.reciprocal(dns, dns)
                    nc.sync.dma_start(out=rc_dram[g0:g0 + 2, :], in_=dns)
                    for h in range(HPC):
                        rows = slice(h * 64, (h + 1) * 64)
                        bcast = strm.tile([128, 1024], F32, tag="bcast")
                        src = rc_dram[g0 + h:g0 + h + 1, :]
                        rbc = bass.AP(
                            tensor=src.tensor,
                            offset=src.offset,
                            ap=[[0, 64]] + list(src.ap)[1:],
                        )
                        nc.sync.dma_start(out=bcast[rows, :], in_=rbc)
                        nc.vector.tensor_mul(
                            o_sb[rows, icol], o_sb[rows, icol], bcast[rows, :]
                        )
                    proj_ready.append((b, ihalf))

            while proj_ready:
                emit_proj(*proj_ready.pop(0))

    nc.finalize()
    return nc


_CACHED = {}


def kernel(x, w_in, w_out, b_out, _trace=False):
    if "nc" not in _CACHED:
        _CACHED["nc"] = _build_nc()
    nc = _CACHED["nc"]

    x2 = np.ascontiguousarray(
        x.reshape(NT, DIM).T.astype(np.float32)
    )  # [DIM, NT]
    in_maps = []
    for c in range(NCORES):
        h0, h1 = HPC * c, HPC * c + 1
        cols = []
        for part in range(3):  # q, k, v
            base = part * DIM
            cols.extend(range(base + h0 * HD, base + h0 * HD + HD))
            cols.extend(range(base + h1 * HD, base + h1 * HD + HD))
        w_in_c = np.ascontiguousarray(w_in[:, cols].astype(np.float32))
        w_out_c = np.ascontiguousarray(
            w_out[128 * c:128 * (c + 1), :].astype(np.float32)
        )
        in_maps.append(
            {
                "xT": x2.astype(ml_dtypes.bfloat16),
                "w_in_c": w_in_c.astype(ml_dtypes.bfloat16),
                "w_out_c": w_out_c.astype(ml_dtypes.bfloat16),
            }
        )

    res = run_bass_kernel_spmd(
        nc, in_maps, core_ids=list(range(NCORES)), trace=_trace
    )
    acc = res.results[0]["po"].astype(np.float64)
    for c in range(1, NCORES):
        acc = acc + res.results[c]["po"].astype(np.float64)
    out = acc.T + b_out.astype(np.float64)
    if _trace:
        kernel.last_result = res
    return np.ascontiguousarray(out.reshape(B, N, DIM).astype(np.float32))



# revision 7
# speedup vs baseline: 1.5200x; 1.5200x over previous
"""Multi-head attention (b=4, n=2048, dim=1024, heads=16) on 8 TRN2 cores.

Sharding: tensor-parallel over heads (2 heads per core) + row-parallel output
projection; host sums the 8 partial outputs and adds the bias.

Schedule: the TRN2 PE drops to a half-speed p-state after any idle gap and
needs 3us of continuous execution to re-ramp, so the kernel keeps the PE
busy end-to-end: attention (scores -> exp -> attnV) is locally exp-bound on
the scalar engine, and the spare PE cycles are filled by interleaving the
next batch's QKV projection matmuls and deferred output-projection matmuls
as rationed filler (plus throwaway dummy matmuls when real filler runs dry).

Per-core math (heads h0=2c, h1=2c+1), one chunk = (batch, query-half, head):
  qkv^T = w_in_c^T @ x^T            (filler, f32 psum, 8-step k chains)
  S^T   = k_h^T.T @ q_h^T           (bf16, K=64, f32 psum)
  E^T   = exp(S^T / 8)              (ACT, no max subtraction: scores ~ N(0,1))
  [o^T; denom] = [v_h | 1].T @ E^T  (bf16, M=65 fuses softmax denominator)
  o_norm^T = o^T * (1/denom)        (DVE recip on [128,16] + DMA broadcast)
  partial^T = w_out_c.T @ o_norm^T  (filler) -> DRAM bf16
"""

import os
import sys
import types

import numpy as np

# NTFF-profile hook shim: container's antenv lacks axon_hooks; harmless if
# tracing is never requested.
if "antenv.axon_hooks" not in sys.modules:
    try:
        from trn_agent_boot.trn_boot import _ntff_profile_via_ctypes

        _m = types.ModuleType("antenv.axon_hooks")
        _h = _ntff_profile_via_ctypes("/opt/axon/libaxon_pjrt.so")
        _m.get_axon_ntff_profile_hook = lambda: _h
        _m.set_axon_ntff_profile_hook = lambda hook: None
        sys.modules["antenv.axon_hooks"] = _m
    except Exception:
        pass

import ml_dtypes

import concourse.bacc as bacc
import concourse.bass as bass
import concourse.mybir as mybir
import concourse.tile as tile
from concourse.bass_utils import run_bass_kernel_spmd
from concourse.masks import make_identity

F32 = mybir.dt.float32
BF16 = mybir.dt.bfloat16

B, N, DIM, HEADS = 4, 2048, 1024, 16
HD = DIM // HEADS          # 64
NCORES = 8
HPC = HEADS // NCORES      # 2 heads per core
NT = B * N                 # 8192 tokens
MQKV = 3 * HPC * HD        # 384 qkv output dims per core
SCALE = HD ** -0.5         # 0.125

KT_TILES = DIM // 128      # 8 k-tiles in the projection contraction
JT = N // 128              # 16 j-tiles per batch
NNB = NT // 1024           # 8 token-blocks for qkv

USE_DMA_TRANSPOSE = False


def _build_nc():
    nc = bacc.Bacc("TRN2", target_bir_lowering=False, debug=False)

    xT = nc.dram_tensor("xT", [DIM, NT], BF16, kind="ExternalInput")
    w_in_c = nc.dram_tensor("w_in_c", [DIM, MQKV], BF16, kind="ExternalInput")
    w_out_c = nc.dram_tensor("w_out_c", [128, DIM], BF16, kind="ExternalInput")
    po = nc.dram_tensor("po", [DIM, NT], BF16, kind="ExternalOutput")
    dn_dram = nc.dram_tensor("dn_dram", [16, 1024], F32)
    rc_dram = nc.dram_tensor("rc_dram", [16, 1024], BF16)

    xT_r = xT.rearrange("(kt p) n -> p kt n", p=128)

    with tile.TileContext(nc) as tc:
        with (
            tc.tile_pool(name="big", bufs=1) as big,
            tc.tile_pool(name="xinp", bufs=3) as xinp,
            tc.tile_pool(name="strm", bufs=2) as strm,
            tc.tile_pool(name="et", bufs=6) as etp,
            tc.tile_pool(name="pout", bufs=4) as poutp,
            tc.tile_pool(name="stp", bufs=2, space="PSUM") as stp,
            tc.tile_pool(name="pop", bufs=1, space="PSUM") as popp,
            tc.tile_pool(name="fillp", bufs=2, space="PSUM") as fillp,
        ):
            # ---- persistent SBUF ----
            QT = big.tile([128, NT], BF16)    # [q_h0(0:64); q_h1(64:128)]^T
            KT = big.tile([128, NT], BF16)
            Vt = big.tile([128, B * JT, 130], BF16)  # [v_h0|1|v_h1|1] per j-tile
            o_sb = big.tile([128, NT], BF16)  # o^T both heads (normed in place)
            w_in_sb = big.tile([128, KT_TILES, MQKV], BF16)
            w_out_sb = big.tile([128, DIM], BF16)

            nc.sync.dma_start(
                out=w_in_sb,
                in_=w_in_c.rearrange("(kt p) m -> p kt m", p=128),
            )
            nc.sync.dma_start(out=w_out_sb, in_=w_out_c[:, :])
            if not USE_DMA_TRANSPOSE:
                ident = big.tile([128, 128], BF16)
                make_identity(nc, ident)
            nc.vector.memset(Vt[:, :, 64], 1.0)
            nc.vector.memset(Vt[:, :, 129], 1.0)

            xin_tiles = {}

            def emit_xin_dma(nb):
                xin = xinp.tile(
                    [128, KT_TILES, 2, 512], BF16, tag="xin", name=f"xin{nb}"
                )
                ncol = slice(nb * 1024, (nb + 1) * 1024)
                src = xT_r[:, :, ncol].rearrange("p kt (a b) -> p kt a b", b=512)
                for k in range(KT_TILES):
                    for a in range(2):
                        nc.sync.dma_start(
                            out=xin[:, k, a, :], in_=src[:, k, a, :]
                        )
                xin_tiles[nb] = xin

            # ---------- filler unit generators ----------
            def qkv_unit(nb, m, a):
                """8-chain projection matmul unit -> QT/KT/Vt columns."""
                pj = fillp.tile([128, 512], F32, tag="fill", name=f"pj{nb}{m}{a}")
                xin = xin_tiles[nb]
                for k in range(KT_TILES):
                    nc.tensor.matmul(
                        pj,
                        w_in_sb[:, k, m * 128:(m + 1) * 128],
                        xin[:, k, a, :],
                        start=(k == 0),
                        stop=(k == KT_TILES - 1),
                    )
                    yield 1
                cols = slice(nb * 1024 + a * 512, nb * 1024 + (a + 1) * 512)
                if m == 0:
                    nc.vector.tensor_copy(QT[:, cols], pj)
                elif m == 1:
                    nc.vector.tensor_copy(KT[:, cols], pj)
                else:
                    vstage = strm.tile([128, 512], BF16, tag="vstage")
                    nc.vector.tensor_copy(vstage, pj)
                    g0 = nb * 8 + a * 4
                    if USE_DMA_TRANSPOSE:
                        nc.sync.dma_start_transpose(
                            out=Vt[:, g0:g0 + 4, 0:64], in_=vstage[0:64, :]
                        )
                        nc.sync.dma_start_transpose(
                            out=Vt[:, g0:g0 + 4, 65:129], in_=vstage[64:128, :]
                        )
                        yield 0
                    else:
                        for c in range(4):
                            tp = fillp.tile(
                                [128, 128], BF16, tag="fill", name=f"tp{g0 + c}"
                            )
                            nc.tensor.transpose(
                                tp, vstage[:, c * 128:(c + 1) * 128], ident
                            )
                            yield 1
                            nc.vector.tensor_copy(Vt[:, g0 + c, 0:64], tp[:, 0:64])
                            nc.vector.tensor_copy(
                                Vt[:, g0 + c, 65:129], tp[:, 64:128]
                            )
                        yield 0

            def proj_unit(bp, ihp, mt, a):
                """One output-projection matmul -> po DRAM (bf16 partial)."""
                i0 = bp * N + ihp * 1024 + a * 512
                pp = fillp.tile(
                    [128, 512], F32, tag="fill", name=f"pp{bp}{ihp}{mt}{a}"
                )
                nc.tensor.matmul(
                    pp,
                    w_out_sb[:, mt * 128:(mt + 1) * 128],
                    o_sb[:, i0:i0 + 512],
                    start=True,
                    stop=True,
                )
                yield 1
                pb = poutp.tile([128, 512], BF16, tag="pout")
                nc.vector.tensor_copy(pb, pp)
                nc.sync.dma_start(
                    out=po[mt * 128:(mt + 1) * 128, i0:i0 + 512], in_=pb
                )
                yield 0

            def dma_unit(fn, *args):
                fn(*args)
                yield 0

            class Filler:
                def __init__(self):
                    self.q = []
                    self.ndummy = 0

                def add(self, gen):
                    self.q.append(gen)

                def pop(self, n, dummy_ok=True):
                    got = 0
                    while got < n:
                        if not self.q:
                            if not dummy_ok:
                                return got
                            # dummy matmul: keeps the PE p-state ramped when
                            # real filler is exhausted; result never read
                            dj = fillp.tile(
                                [128, 512], F32, tag="fill",
                                name=f"dj{self.ndummy}",
                            )
                            self.ndummy += 1
                            nc.tensor.matmul(
                                dj, w_out_sb[:, 0:128], w_out_sb[:, 0:512],
                                start=True, stop=True,
                            )
                            got += 1
                            continue
                        try:
                            got += next(self.q[0])
                        except StopIteration:
                            self.q.pop(0)
                    return got

            filler = Filler()

            # ---------- attention chunk machinery ----------
            chunks = [
                (b, ih, h)
                for b in range(B)
                for ih in range(2)
                for h in range(HPC)
            ]

            def emit_scores(b, ih, h, jt, ci):
                hp = slice(h * 64, (h + 1) * 64)
                i0 = b * N + ih * 1024
                jcol = slice(b * N + jt * 128, b * N + jt * 128 + 128)
                st = stp.tile(
                    [128, 2, 512], F32, tag="st", name=f"st{ci}_{jt}"
                )
                for a in range(2):
                    nc.tensor.matmul(
                        st[:, a, :],
                        KT[hp, jcol],
                        QT[hp, i0 + a * 512:i0 + (a + 1) * 512],
                        start=True,
                        stop=True,
                    )
                et = etp.tile([128, 1024], BF16, tag="et", name="et")
                nc.scalar.activation(
                    et,
                    st.rearrange("p a b -> p (a b)"),
                    mybir.ActivationFunctionType.Exp,
                    scale=SCALE,
                )
                return et

            def emit_attnv(b, h, jp, et, po_t):
                for a in range(2):
                    nc.tensor.matmul(
                        po_t[:, a, :],
                        Vt[:, b * JT + jp, h * 65:h * 65 + 65],
                        et[:, a * 512:(a + 1) * 512],
                        start=(jp == 0),
                        stop=(jp == JT - 1),
                    )

            def emit_drain(b, ih, h, po_t):
                i0 = b * N + ih * 1024
                icol = slice(i0, i0 + 1024)
                seg = b * 4 + ih * 2 + h
                po_f = po_t.rearrange("p a b -> p (a b)")
                if h == 0:
                    nc.vector.tensor_copy(o_sb[0:64, icol], po_f[0:64, :])
                else:
                    h1s = strm.tile([64, 1024], BF16, tag="h1s")
                    nc.vector.tensor_copy(h1s, po_f[0:64, :])
                    nc.sync.dma_start(out=o_sb[64:128, icol], in_=h1s)
                dnrow = strm.tile([1, 1024], F32, tag="dnrow")
                nc.vector.tensor_copy(dnrow, po_f[64:65, :])
                nc.sync.dma_start(out=dn_dram[seg:seg + 1, :], in_=dnrow)

            def emit_normalize(b, ih):
                i0 = b * N + ih * 1024
                icol = slice(i0, i0 + 1024)
                g0 = b * 4 + ih * 2
                dns = strm.tile([128, 16], F32, tag="dns")
                for hh in range(2):
                    nc.sync.dma_start(
                        out=dns[:, hh * 8:(hh + 1) * 8],
                        in_=dn_dram[g0 + hh:g0 + hh + 1, :].rearrange(
                            "o (p a) -> (o p) a", p=128
                        ),
                    )
                with nc.allow_low_precision(reason="softmax denom recip"):
                    nc.vector.reciprocal(dns, dns)
                rcc = strm.tile([128, 16], BF16, tag="rcc")
                nc.vector.tensor_copy(rcc, dns)
                for hh in range(2):
                    nc.sync.dma_start(
                        out=rc_dram[g0 + hh:g0 + hh + 1, :].rearrange(
                            "o (p a) -> (o p) a", p=128
                        ),
                        in_=rcc[:, hh * 8:(hh + 1) * 8],
                    )
                for hh in range(2):
                    rows = slice(hh * 64, (hh + 1) * 64)
                    bcast = strm.tile([128, 1024], BF16, tag="bcast")
                    src = rc_dram[g0 + hh:g0 + hh + 1, :]
                    rbc = bass.AP(
                        tensor=src.tensor,
                        offset=src.offset,
                        ap=[[0, 64]] + list(src.ap)[1:],
                    )
                    nc.sync.dma_start(out=bcast[rows, :], in_=rbc)
                    nc.vector.tensor_mul(
                        o_sb[rows, icol], o_sb[rows, icol], bcast[rows, :]
                    )

            # ---------- build filler queue: prologue qkv for b0 ----------
            emit_xin_dma(0)
            emit_xin_dma(1)
            for nb in range(2):
                for m in range(3):
                    for a in range(2):
                        filler.add(qkv_unit(nb, m, a))
            filler.pop(10 ** 6, dummy_ok=False)  # prologue: drain all of b0

            # queue qkv for batches 1..3 (consumed as filler during
            # attention); keep xin DMAs 2 blocks ahead of their consumers
            emit_xin_dma(2)
            emit_xin_dma(3)
            for nb in range(2, NNB):
                if nb + 2 < NNB:
                    filler.add(dma_unit(emit_xin_dma, nb + 2))
                for m in range(3):
                    for a in range(2):
                        filler.add(qkv_unit(nb, m, a))

            # ---------- main loop ----------
            for ci, (b, ih, h) in enumerate(chunks):
                ets = {}
                po_t = popp.tile(
                    [65, 2, 512], F32, tag="po", name=f"po{ci}"
                )
                # filler rate: qkv supply is 1.5/slot for batches 0-2; the
                # proj backlog drains during batch 3
                rate = 1.5 if ci < 12 else 1.75
                acc = 2.5  # chunk-start boost: covers prev drain latency
                for jt in range(JT):
                    ets[jt] = emit_scores(b, ih, h, jt, ci)
                    if jt >= 2:
                        emit_attnv(b, h, jt - 2, ets.pop(jt - 2), po_t)
                    acc += rate
                    npop = int(acc)
                    if npop:
                        filler.pop(npop)
                        acc -= npop
                emit_attnv(b, h, JT - 2, ets.pop(JT - 2), po_t)
                filler.pop(1)
                emit_attnv(b, h, JT - 1, ets.pop(JT - 1), po_t)
                emit_drain(b, ih, h, po_t)
                if h == 1:
                    emit_normalize(b, ih)
                    if ci < len(chunks) - 1:
                        for mt in range(DIM // 128):
                            for a in range(2):
                                filler.add(proj_unit(b, ih, mt, a))
                filler.pop(2)

            # epilogue: last chunk's projection
            for mt in range(DIM // 128):
                for a in range(2):
                    filler.add(proj_unit(B - 1, 1, mt, a))
            filler.pop(10 ** 6, dummy_ok=False)

    nc.finalize()
    return nc


_CACHED = {}


def kernel(x, w_in, w_out, b_out, _trace=False):
    if "nc" not in _CACHED:
        _CACHED["nc"] = _build_nc()
    nc = _CACHED["nc"]

    x2 = np.ascontiguousarray(
        x.reshape(NT, DIM).T.astype(np.float32)
    )  # [DIM, NT]
    in_maps = []
    for c in range(NCORES):
        h0, h1 = HPC * c, HPC * c + 1
        cols = []
        for part in range(3):  # q, k, v
            base = part * DIM
            cols.extend(range(base + h0 * HD, base + h0 * HD + HD))
            cols.extend(range(base + h1 * HD, base + h1 * HD + HD))
        w_in_cc = np.ascontiguousarray(w_in[:, cols].astype(np.float32))
        w_out_cc = np.ascontiguousarray(
            w_out[128 * c:128 * (c + 1), :].astype(np.float32)
        )
        in_maps.append(
            {
                "xT": x2.astype(ml_dtypes.bfloat16),
                "w_in_c": w_in_cc.astype(ml_dtypes.bfloat16),
                "w_out_c": w_out_cc.astype(ml_dtypes.bfloat16),
            }
        )

    res = run_bass_kernel_spmd(
        nc, in_maps, core_ids=list(range(NCORES)), trace=_trace
    )
    acc = res.results[0]["po"].astype(np.float64)
    for c in range(1, NCORES):
        acc = acc + res.results[c]["po"].astype(np.float64)
    out = acc.T + b_out.astype(np.float64)
    if _trace:
        kernel.last_result = res
    return np.ascontiguousarray(out.reshape(B, N, DIM).astype(np.float32))
